# revision 28
# baseline (speedup 1.0000x reference)
"""Trainium2 Bass kernel for ProbLinear (Bayesian linear layer, sampled weights).

Computes:
    W    = weight_mu + softplus(weight_rho) * eps_w          [OUT_F, IN_F]
    b    = bias_mu + softplus(bias_rho) * eps_b              [OUT_F]
    out  = x @ W.T + b                                       [TOKENS, OUT_F]

Sharding across 8 NeuronCores: column-parallel — each core owns 512 of the
4096 out_features and all 8192 tokens. Inputs are pre-transposed to K-major
and cast to bf16 on the host (a free layout/precision choice during
sharding; bf16 matmul runs at the same full PE rate as f32r and halves all
HBM traffic; ~1e-2 rel-err budget has plenty of room). The device samples
W^T = mu^T + softplus(rho^T)*eps^T elementwise in K-major layout — no
on-chip transposes — and runs the K-accumulated matmul with W^T stationary
and x^T moving, accumulating out^T[o, tokens] tiles in PSUM. The
per-out-feature bias rides the PSUM->SBUF eviction for free as the
per-partition bias operand of an ACT Identity op.

Startup is the only non-PE time and is bound by delivering ~20MB (12MB of
W inputs + the first two 4MB token slabs) from HBM: slab0 rides the GpSimd
DMA queue while the W chunks stream in PE-consumption order down the Sync
queue with slab1's 1MB quarter-tiles interleaved at the positions the ramp
needs them; steady-state slabs queue on Sync BEHIND all W DMAs so W has
strict bandwidth priority. The first two token slabs accumulate
W-chunk-major across all 8 PSUM banks (slab1's K-order rotated to start at
chunk ROT, when its data has landed — PSUM accumulation over K is
order-free, so this is exact), and the PE starts on a single-k-tile first
chunk ~15us in, then runs gap-free at ~99% MM occupancy.

Self-contained: hardcodes shapes, builds + caches the Bass program, shards
inputs on the host, runs via run_bass_kernel_spmd, reassembles full output.
"""
import numpy as np
from contextlib import ExitStack

import ml_dtypes

import concourse.bass as bass
import concourse.mybir as mybir
import concourse.tile as tile
from concourse.bass_utils import run_bass_kernel_spmd

# ----------------------------------------------------------------------------
# Workaround for this walrus build: only 1 sem wait per instruction is
# accepted by some codegen paths. After Tile scheduling, hoist excess waits
# onto same-engine NoOps inserted right before the offending instruction.
# ----------------------------------------------------------------------------
_MAX_WAITS = 1


def _split_excess_waits(nc):
    for f in nc.m.functions:
        for bb in f.blocks:
            insts = bb.instructions
            i = 0
            while i < len(insts):
                inst = insts[i]
                si = inst.sync_info
                if si is not None and len(si.on_wait) > _MAX_WAITS:
                    waits = list(si.on_wait)
                    excess, keep = waits[:-_MAX_WAITS], waits[-_MAX_WAITS:]
                    si.on_wait = keep
                    pos = i
                    for j in range(0, len(excess), _MAX_WAITS):
                        chunk = excess[j:j + _MAX_WAITS]
                        nop = mybir.InstNoOp(
                            name=f"{inst.name}-waitsplit-{j}", ins=[], outs=[]
                        )
                        nop.engine = inst.engine
                        nop.sync_info = mybir.SyncInfo(on_wait=chunk, on_update=[])
                        nc.register_instruction(nop, overwrite=True)
                        insts.insert(pos, nop)
                        pos += 1
                        i += 1
                i += 1


if not getattr(tile.TileContext, "_waitsplit_patched", False):
    _orig_exit = tile.TileContext.__exit__

    def _patched_exit(self, exc_type, exc_val, exc_tb):
        res = _orig_exit(self, exc_type, exc_val, exc_tb)
        if exc_type is None:
            _split_excess_waits(self.nc)
        return res

    tile.TileContext.__exit__ = _patched_exit
    tile.TileContext._waitsplit_patched = True

# ----------------------------------------------------------------------------
# Problem shapes / sharding
# ----------------------------------------------------------------------------
TOKENS, IN_F, OUT_F = 8192, 4096, 4096
N_CORES = 8
O_C = OUT_F // N_CORES           # 512 out features per core
KT = IN_F // 128                 # 32 contraction k-tiles
# W sampling chunks as (first k-tile, n k-tiles): two single-k-tile chunks
# cut the PE's first-weights latency, then 2-k-tile chunks keep the sampled
# stream arriving at the ramp's consumption pace.
CH = [(0, 1), (1, 1)] + [(2 + 2 * i, 2) for i in range(15)]
TS = 512                         # token slab width (= PSUM bank free dim)
NSLAB = TOKENS // TS             # 16
NOT = O_C // 128                 # 4 o-tiles per core
QKT = 8                          # k-tiles per x quarter-tile (1MB pieces)
NQ = KT // QKT                   # 4 quarters per slab
ROT = 5                          # slab1 joins at CH[ROT] (k-tile 8), wraps

F32 = mybir.dt.float32
BF16 = mybir.dt.bfloat16
AF = mybir.ActivationFunctionType
# The walrus act tables have no softplus set; exp+ln+identity all live in
# one set (natural_log_exp_and_others) so the kernel loads one ACT table.


def _kview(ap):
    """[K, N] dram AP -> [128, KT_sub, N] with partition = k % 128."""
    return ap.rearrange("(kt p) t -> p kt t", p=128)


def _build_program():
    nc = bass.Bass()
    xT_d = nc.declare_dram_parameter("xT", [IN_F, TOKENS], BF16, isOutput=False)
    wmu_d = nc.declare_dram_parameter("wmuT", [IN_F, O_C], BF16, isOutput=False)
    wrho_d = nc.declare_dram_parameter("wrhoT", [IN_F, O_C], BF16, isOutput=False)
    weps_d = nc.declare_dram_parameter("wepsT", [IN_F, O_C], BF16, isOutput=False)
    bmu_d = nc.declare_dram_parameter("bmu", [O_C], F32, isOutput=False)
    brho_d = nc.declare_dram_parameter("brho", [O_C], F32, isOutput=False)
    beps_d = nc.declare_dram_parameter("beps", [O_C], F32, isOutput=False)
    out_d = nc.declare_dram_parameter("outT", [O_C, TOKENS], BF16, isOutput=True)

    xv = _kview(xT_d[:, :])
    wmuv = _kview(wmu_d[:, :])
    wrhov = _kview(wrho_d[:, :])
    wepsv = _kview(weps_d[:, :])
    ov = out_d[:, :].rearrange("(ot p) t -> p ot t", p=128)

    with tile.TileContext(nc) as tc, ExitStack() as ctx:
        const = ctx.enter_context(tc.tile_pool(name="const", bufs=1))
        wpool = ctx.enter_context(tc.tile_pool(name="wpool", bufs=1))
        stage = ctx.enter_context(tc.tile_pool(name="stage", bufs=6))
        xpool = ctx.enter_context(tc.tile_pool(name="xpool", bufs=2 * NQ))
        opool = ctx.enter_context(tc.tile_pool(name="opool", bufs=6))
        mmpsum = ctx.enter_context(tc.tile_pool(name="mmpsum", bufs=1, space="PSUM"))

        # ------------------------------------------------------------------
        # W^T sampling, one chunk (nkt k-tiles) at a time:
        #   wTc[ci][p, kt, o] = W[o, (k0+kt)*128 + p]  as bf16
        # Chunk 0 is emitted before anything else: its rho is the very
        # first Sync DMA, and its Exp/Ln pin the ACT table load and the
        # const-AP memsets to the head of every engine queue.
        # ------------------------------------------------------------------
        wTc = [None] * len(CH)
        stage_mu0 = []

        def emit_chunk(ci):
            k0, nkt = CH[ci]
            rho = stage.tile([128, nkt, O_C], BF16, tag="stg_r", name="rho")
            eps = stage.tile([128, nkt, O_C], BF16, tag="stg_e", name="eps")
            mu = stage.tile([128, nkt, O_C], BF16, tag="stg_m", name="mu")
            nc.sync.dma_start(rho[:], wrhov[:, k0:k0 + nkt])
            nc.sync.dma_start(eps[:], wepsv[:, k0:k0 + nkt])
            nc.sync.dma_start(mu[:], wmuv[:, k0:k0 + nkt])
            if ci in S1_AT:
                q = S1_AT[ci]
                nc.sync.dma_start(
                    slab1[q][:], xv[:, q * QKT:(q + 1) * QKT, TS:2 * TS]
                )
            if ci == 0:
                # schedule chunk0's softplus as a unit: the list scheduler
                # otherwise slots exp1 between exp0 and ln0 on ACT, and
                # ln0 then stalls on rho1's arrival instead of finishing
                with tc.tile_critical():
                    nc.scalar.activation(rho[:], rho[:], AF.Exp)
                    nc.scalar.activation(rho[:], rho[:], AF.Ln, bias=1.0)
            else:
                nc.scalar.activation(rho[:], rho[:], AF.Exp)
                nc.scalar.activation(rho[:], rho[:], AF.Ln, bias=1.0)
            wt = wpool.tile([128, nkt, O_C], BF16, tag=f"wT{ci}", name=f"wT{ci}")
            nc.vector.tensor_mul(eps[:], eps[:], rho[:])
            nc.vector.tensor_add(wt[:], eps[:], mu[:])
            wTc[ci] = wt
            if ci == 0:
                stage_mu0.append(mu)

        # slab1's quarters are DMA'd from inside emit_chunk, interleaved
        # into the Sync stream in the rotated order the ramp consumes them
        # — arrival order matches PE consumption order. Tiles are created
        # after slab0's (below) so xpool slot rotation frees slab0 first.
        S1_AT = {3: 1, 7: 2, 11: 3, 16: 0}

        emit_chunk(0)
        # ~40 discarded matmuls reading chunk0's mu staging tile (the
        # earliest bytes in SBUF, ~9us; read-only, so no WAR with the
        # sampling chain) keep the PE busy through the weight wait and
        # enter the ramp with HAM already at full rate.
        warm = mmpsum.tile([128, TS], F32, tag="ps7", name="warm")
        mu0 = stage_mu0[0]
        for i in range(40):
            nc.tensor.matmul(
                warm[:], mu0[:, 0, (i % 4) * 128:(i % 4 + 1) * 128], mu0[:, 0],
                start=(i == 0), stop=(i == 39),
            )

        # ------------------------------------------------------------------
        # x slabs: NQ quarter-tiles of [128, QKT, TS] bf16, K-major from
        # DRAM. The two ramp slabs ride the GpSimd queue (concurrent with
        # the W stream on Sync); steady-state slabs ride the Sync queue
        # BEHIND all W DMAs — the in-order queue gives W strict priority
        # for HBM bandwidth during the startup window.
        # ------------------------------------------------------------------
        def load_slab(s, eng):
            qs = []
            for q in range(NQ):
                t = xpool.tile([128, QKT, TS], BF16, tag="xq")
                eng.dma_start(
                    t[:], xv[:, q * QKT:(q + 1) * QKT, s * TS:(s + 1) * TS]
                )
                qs.append(t)
            return qs

        slab0 = load_slab(0, nc.gpsimd)
        slab1 = [
            xpool.tile([128, QKT, TS], BF16, tag="xq", name=f"s1q{q}")
            for q in range(NQ)
        ]
        slabs = {0: slab0, 1: slab1}

        for ci in range(1, len(CH)):
            emit_chunk(ci)
        kmap = {}
        for ci, (k0, nkt) in enumerate(CH):
            for kt in range(nkt):
                kmap[k0 + kt] = (ci, kt)

        # ------------------------------------------------------------------
        # Bias: [128, NOT] f32 column table; bias[:, ot] feeds the ACT
        # Identity per-partition bias during PSUM eviction. Emitted
        # after the W chunks: its Sync DMAs and ACT ops stay clear of the
        # startup-critical W pipeline (first needed at the first group
        # close, ~75us in).
        # ------------------------------------------------------------------
        bias_sb = const.tile([128, NOT], F32)
        bmu_t = const.tile([128, NOT], F32)
        beps_t = const.tile([128, NOT], F32)
        nc.sync.dma_start(bias_sb[:], brho_d[:].rearrange("(c p) -> p c", p=128))
        nc.sync.dma_start(bmu_t[:], bmu_d[:].rearrange("(c p) -> p c", p=128))
        nc.sync.dma_start(beps_t[:], beps_d[:].rearrange("(c p) -> p c", p=128))
        nc.scalar.activation(bias_sb[:], bias_sb[:], AF.Exp)
        nc.scalar.activation(bias_sb[:], bias_sb[:], AF.Ln, bias=1.0)
        nc.vector.tensor_mul(bias_sb[:], bias_sb[:], beps_t[:])
        nc.vector.tensor_add(bias_sb[:], bias_sb[:], bmu_t[:])


        # Steady slabs 2 and 3 queue on Sync directly behind the last W DMA
        # (before any epilogue out-DMA can head-of-line block the queue).
        preloaded = {2: load_slab(2, nc.sync), 3: load_slab(3, nc.sync)}

        # ------------------------------------------------------------------
        # Matmul: out^T[o_tile, tokens] += wT[k, o_tile].T @ xT[k, tokens]
        # PSUM group g -> bank tag g % 8; epilogue = ACT Identity with
        # per-partition bias, bf16 out, DMA on the Sync queue.
        # ------------------------------------------------------------------
        pss = {}

        def open_group(g):
            pss[g] = mmpsum.tile([128, TS], F32, tag=f"ps{g % 8}", name=f"ps{g % 8}")

        def close_group(g, s, ot):
            ob = opool.tile([128, TS], BF16, tag="ob")
            nc.scalar.activation(
                ob[:], pss[g][:], AF.Identity, bias=bias_sb[:, ot:ot + 1]
            )
            nc.sync.dma_start(ov[:, ot, s * TS:(s + 1) * TS], ob[:])
            del pss[g]

        def mm_block(ci, si, is_start, is_stop):
            k0, nkt = CH[ci]
            for ot in range(NOT):
                for kt in range(nkt):
                    k = k0 + kt
                    nc.tensor.matmul(
                        pss[si * NOT + ot][:],
                        wTc[ci][:, kt, ot * 128:(ot + 1) * 128],
                        slabs[si][k // QKT][:, k % QKT],
                        start=(is_start and kt == 0),
                        stop=(is_stop and kt == nkt - 1),
                    )

        # Ramp: slabs 0 and 1 accumulate chunk-major across all 8 banks so
        # the PE starts on chunk 0 and rides the sampling wave; slab1's
        # K-order starts at chunk ROT (when its DMA has landed) and wraps.
        # PSUM accumulation over K is order-free, so this is exact.
        NCHV = len(CH)
        for si in range(2):
            for ot in range(NOT):
                open_group(si * NOT + ot)
        for ci in range(NCHV):
            mm_block(ci, 0, is_start=(ci == 0), is_stop=(ci == NCHV - 1))
            if ci >= ROT:
                mm_block(ci, 1, is_start=(ci == ROT), is_stop=False)
        for ci in range(ROT):
            mm_block(ci, 1, is_start=False, is_stop=(ci == ROT - 1))
        for si in range(2):
            for ot in range(NOT):
                close_group(si * NOT + ot, si, ot)

        # Steady state: k-major per (slab, o_tile) group. Slab s+2 is
        # prefetched at iteration s so its DMAs sit on the sync queue
        # ahead of out-DMAs that block on unfinished matmuls.
        for s in range(2, NSLAB):
            if s + 2 < NSLAB:
                preloaded[s + 2] = load_slab(s + 2, nc.sync)
            qs = preloaded.pop(s)
            for ot in range(NOT):
                g = s * NOT + ot
                open_group(g)
                for k in range(KT):
                    ci, kt = kmap[k]
                    nc.tensor.matmul(
                        pss[g][:],
                        wTc[ci][:, kt, ot * 128:(ot + 1) * 128],
                        qs[k // QKT][:, k % QKT],
                        start=(k == 0),
                        stop=(k == KT - 1),
                    )
                close_group(g, s, ot)

    return nc


_PROGRAM = None
NPBF16 = ml_dtypes.bfloat16


def kernel(x, weight_mu, weight_rho, bias_mu, bias_rho, eps_w, eps_b):
    global _PROGRAM
    if _PROGRAM is None:
        _PROGRAM = _build_program()
    nc = _PROGRAM

    xT = np.ascontiguousarray(np.asarray(x, dtype=np.float32).astype(NPBF16).T)
    wmuT = np.asarray(weight_mu, dtype=np.float32).astype(NPBF16).T
    wrhoT = np.asarray(weight_rho, dtype=np.float32).astype(NPBF16).T
    wepsT = np.asarray(eps_w, dtype=np.float32).astype(NPBF16).T
    bias_mu = np.ascontiguousarray(np.asarray(bias_mu, dtype=np.float32))
    bias_rho = np.ascontiguousarray(np.asarray(bias_rho, dtype=np.float32))
    eps_b = np.ascontiguousarray(np.asarray(eps_b, dtype=np.float32))

    in_maps = []
    for c in range(N_CORES):
        os_, oe = c * O_C, (c + 1) * O_C
        in_maps.append({
            "xT": xT,
            "wmuT": np.ascontiguousarray(wmuT[:, os_:oe]),
            "wrhoT": np.ascontiguousarray(wrhoT[:, os_:oe]),
            "wepsT": np.ascontiguousarray(wepsT[:, os_:oe]),
            "bmu": bias_mu[os_:oe],
            "brho": bias_rho[os_:oe],
            "beps": eps_b[os_:oe],
        })

    res = run_bass_kernel_spmd(nc, in_maps, list(range(N_CORES)))
    kernel.last_results = res

    outT = np.concatenate([res.results[c]["outT"] for c in range(N_CORES)], axis=0)
    return np.ascontiguousarray(outT.T).astype(np.float32)
